# revision 6
# baseline (speedup 1.0000x reference)
"""Trainium2 Bass kernel for nn_AttentionBlock (pre-LN transformer block).

Reference math (fp32):
    h  = LN(x, g1, b1); q,k,v = h@wq, h@wk, h@wv  (heads=8, d=64)
    att = softmax(q k^T / sqrt(512)) v;  x2 = att@wo + bo + x
    out = silu(LN(x2,g2,b2)@w1 + bf1)@w2 + bf2 + x2

Sharding over 8 NeuronCores: core c handles batch n=c//2, query-half c%2
(1024 queries). K/V are computed per-core over the full 2048-token sequence
(duplicated within each batch pair; softmax over keys is permutation
invariant, so each core receives the sequence rotated so its own tokens come
first — fixed offsets, single shared NEFF, no collectives).

v10 layout/scheduling notes:
  - feature-major on-chip ([feature partitions, token free-dim]); host passes
    x pre-transposed per core, output transposed back on host.
  - all per-feature vectors ride in one packed [128, 56] fp32 "vecs" DMA.
  - phase-C weight prefetches are gated on an early phase-A tile so the Tile
    scheduler cannot hoist them to t=0 and starve the critical-path DMAs.
  - attention runs qb-outer so phase C for query-half 0 overlaps attention of
    half 1; softmax exp is split: even heads exact on ScalarE, odd heads via
    a bf16-domain Schraudolph exp on VectorE (int16 bit trick).
  - silu uses Tanh (same activation table set as Exp — no table thrash);
    both layernorms use the Newton-rsqrt DVE path (no scalar Sqrt).
"""

import os
from contextlib import ExitStack

import numpy as np

import concourse.bass as bass
import concourse.tile as tile
from concourse import bacc, mybir
from concourse._compat import with_exitstack
from concourse.bass_utils import run_bass_kernel_spmd

F32 = mybir.dt.float32
F32R = mybir.dt.float32r
BF16 = mybir.dt.bfloat16
I32 = mybir.dt.int32
I16 = mybir.dt.int16
AF = mybir.ActivationFunctionType
ALU = mybir.AluOpType

P = 128          # partitions
E = 512          # embedding
NE = E // P      # 4 feature tiles
L = 2048         # sequence length
LQ = 1024        # queries per core
TB = 512         # token block
NTB = L // TB    # 4
NQB = LQ // TB   # 2
H = 8            # heads
D = 64           # head dim
NPAIR = H // 2   # 4 head pairs
NJT = L // P     # 16 key tiles
FF = 2048
NF1 = FF // P    # 16
SCALE = float(E) ** -0.5
EPS = 1e-5
MAGIC = 0x5F3759DF
# bf16-domain Schraudolph exp: bf16_bits(exp(s*SCALE)) ~ EXP_A*s + EXP_B
EXP_A = (128.0 / float(np.log(2.0))) * SCALE
EXP_B = 127.0 * 128.0 - 5.49
N_CORES = 8
KERNEL_VERSION = 10  # bump on every kernel change: busts the neuron compile cache

# packed vecs column offsets
VG1, VB1, VG2, VB2, VBO, VBF2, VBF1, VBF1H = 0, 4, 8, 12, 16, 20, 24, 40
VECS_W = 56


@with_exitstack
def _emit(ctx: ExitStack, tc: tile.TileContext, d):
    nc = tc.nc
    xT = d["xT"]          # [E, L]
    outT = d["outT"]      # [E, LQ]

    pers = ctx.enter_context(tc.tile_pool(name="pers", bufs=1))

    # constants + packed per-feature vectors
    one_f = pers.tile([P, 1], F32)
    nc.vector.memset(one_f, 1.0)
    inv_e_f = pers.tile([P, 1], F32)
    nc.vector.memset(inv_e_f, 1.0 / E)
    one_col = pers.tile([P, 1], BF16)            # 1/E for mean matmuls
    nc.vector.tensor_copy(out=one_col, in_=inv_e_f)
    magic = pers.tile([1, 1], I32)
    nc.vector.memset(magic, MAGIC)
    vecs = pers.tile([P, VECS_W], F32)
    nc.sync.dma_start(vecs, d["vecs"])

    ONORM = pers.tile([P, NE, LQ], BF16)         # attention out^T, normalized

    def layernorm(rows, bcast, sqpool, mu_ps, ss_ps, x_slices, out_slices,
                  gcol, bcol):
        """x slices: NE aps of [P, TB] (feature-major), fp32 or bf16. Stats via
        ones-matmuls (partition reduction), rstd via DVE Newton rsqrt,
        per-token affine broadcast via GPSIMD; apply in bf16 (2x DVE)."""
        xbs = []
        for et in range(NE):
            if x_slices[et].dtype == BF16:
                xb = x_slices[et]
            else:
                xb = sqpool.tile([P, TB], BF16, tag="xb", bufs=4)
                nc.vector.tensor_copy(out=xb, in_=x_slices[et])
            xbs.append(xb)
            sq = sqpool.tile([P, TB], BF16, tag="sq", bufs=2)
            nc.vector.tensor_mul(sq, xb, xb)
            nc.tensor.matmul(mu_ps, one_col, xb,
                             start=(et == 0), stop=(et == NE - 1))
            nc.tensor.matmul(ss_ps, one_col, sq,
                             start=(et == 0), stop=(et == NE - 1))
        mur = rows.tile([1, TB], F32, tag="mur", bufs=1)
        v = rows.tile([1, TB], F32, tag="var", bufs=1)
        y = rows.tile([1, TB], F32, tag="y", bufs=1)
        t = rows.tile([1, TB], F32, tag="t", bufs=1)
        b = rows.tile([1, TB], F32, tag="b", bufs=1)
        nc.vector.tensor_copy(out=mur, in_=mu_ps)
        nc.vector.tensor_mul(v, mur, mur)
        nc.vector.tensor_tensor(out=v, in0=ss_ps, in1=v, op=ALU.subtract)
        nc.vector.tensor_scalar(out=v, in0=v, scalar1=EPS, scalar2=None,
                                op0=ALU.add)
        # rstd = rsqrt(v): fast-inverse-sqrt seed + Newton steps (DVE only)
        nc.vector.tensor_scalar(out=y.bitcast(I32), in0=v.bitcast(I32),
                                scalar1=1, scalar2=None,
                                op0=ALU.logical_shift_right)
        nc.vector.tensor_tensor(out=y.bitcast(I32),
                                in0=magic.to_broadcast([1, TB]),
                                in1=y.bitcast(I32), op=ALU.subtract)
        for _ in range(2):
            nc.vector.tensor_mul(t, y, y)
            nc.vector.tensor_mul(t, t, v)
            nc.vector.tensor_scalar(out=t, in0=t, scalar1=-0.5,
                                    scalar2=1.5,
                                    op0=ALU.mult, op1=ALU.add)
            nc.vector.tensor_mul(y, y, t)
        nc.vector.tensor_mul(b, mur, y)
        yb = rows.tile([1, TB], BF16, tag="yb", bufs=1)
        bb = rows.tile([1, TB], BF16, tag="bb16", bufs=1)
        nc.vector.tensor_copy(out=yb, in_=y)
        nc.vector.tensor_copy(out=bb, in_=b)
        a_b = bcast.tile([P, TB], BF16, tag="ab", bufs=2)
        b_b = bcast.tile([P, TB], BF16, tag="bb", bufs=2)
        nc.gpsimd.partition_broadcast(a_b, yb)
        nc.gpsimd.partition_broadcast(b_b, bb)
        for et in range(NE):
            o = out_slices[et]          # bf16; all-bf16 ops for 2x DVE
            sc = sqpool.tile([P, TB], BF16, tag="lnsc", bufs=2)
            nc.vector.tensor_mul(sc, xbs[et], a_b)
            nc.vector.tensor_tensor(out=sc, in0=sc, in1=b_b,
                                    op=ALU.subtract)
            nc.vector.tensor_scalar(out=o, in0=sc,
                                    scalar1=gcol[:, et:et + 1],
                                    scalar2=bcol[:, et:et + 1],
                                    op0=ALU.mult, op1=ALU.add)

    # Phase-C operands, DMA'd mid-phase-A (gated below so the scheduler
    # cannot start them at t=0 ahead of the critical-path DMAs)
    pre = ctx.enter_context(tc.tile_pool(name="prefetch", bufs=1))
    wo_sb = pre.tile([P, NE, E], BF16)
    w1_sb = pre.tile([P, NE, FF], BF16)
    w2_sb = pre.tile([P, NF1, E], BF16)

    # ---------------- Phase A: LN1 + Q/K/V projections -------------------
    pab = ctx.enter_context(tc.tile_pool(name="qkv_pers", bufs=1))
    KT = pab.tile([P, NE, L], BF16)          # K^T  (feature-major)
    QT = pab.tile([P, NE, LQ], BF16)         # Q^T
    V = pab.tile([P, NJT, H, D + 1], BF16)   # V token-major + ones column
    xt0 = pab.tile([P, NE, 2 * TB], BF16)    # sb0 tokens: LN1 input + residual
    nc.vector.tensor_copy(out=V[:, :, :, D:D + 1],
                          in_=one_f.unsqueeze(1).unsqueeze(1)
                          .to_broadcast([P, NJT, H, 1]))

    with tc.tile_pool(name="wa", bufs=1) as wa, \
            tc.tile_pool(name="xa", bufs=2) as xa, \
            tc.tile_pool(name="rows_a", bufs=1) as rows_a, \
            tc.tile_pool(name="bc_a", bufs=1) as bc_a, \
            tc.tile_pool(name="ps_a", bufs=1, space="PSUM") as ps_a:
        wq_sb = wa.tile([P, NE, E], BF16, tag="wq")
        wk_sb = wa.tile([P, NE, E], BF16, tag="wk")
        wv_sb = wa.tile([P, NE, E], BF16, tag="wv")
        for t, name in ((wk_sb, "wk"), (wq_sb, "wq"), (wv_sb, "wv")):
            nc.sync.dma_start(
                t, d[name].rearrange("(ko p) f -> p ko f", p=P))

        for sb in range(NTB // 2):      # super-blocks of 1024 tokens
            xn = xa.tile([P, NE, 2 * TB], BF16, tag="xn", bufs=2)
            xt = xt0 if sb == 0 else xa.tile([P, NE, 2 * TB], BF16,
                                             tag="xa", bufs=1)
            nc.sync.dma_start(
                xt, xT[:, sb * 2 * TB:(sb + 1) * 2 * TB].rearrange(
                    "(et p) t -> p et t", p=P))
            for half in range(2):
                hs_ = slice(half * TB, (half + 1) * TB)
                mu_ps = ps_a.tile([1, TB], F32, tag="a", bufs=3)
                ss_ps = ps_a.tile([1, TB], F32, tag="a", bufs=3)
                layernorm(rows_a, bc_a, xa,
                          mu_ps, ss_ps,
                          [xt[:, et, hs_] for et in range(NE)],
                          [xn[:, et, hs_] for et in range(NE)],
                          vecs[:, VG1:VG1 + NE], vecs[:, VB1:VB1 + NE])
            # K^T (and Q^T for this core's token super-block)
            for m in range(NE):
                for half in range(2):
                    hs_ = slice(half * TB, (half + 1) * TB)
                    pk = ps_a.tile([P, TB], F32, tag="a", bufs=3)
                    for et in range(NE):
                        nc.tensor.matmul(pk, wk_sb[:, et, m * P:(m + 1) * P],
                                         xn[:, et, hs_],
                                         start=(et == 0), stop=(et == NE - 1))
                    ks_ = slice(sb * 2 * TB + half * TB,
                                sb * 2 * TB + (half + 1) * TB)
                    nc.scalar.activation(KT[:, m, ks_], pk, AF.Copy)
            if sb == 0:
                # gated phase-C prefetches: the tiny dependency write keeps
                # the 6.5MB of DMA off the t=0 critical path
                for pt_, src in (
                        (wo_sb, d["wo"].rearrange("(ko p) f -> p ko f", p=P)),
                        (w1_sb, d["w1"].rearrange("(ko p) f -> p ko f", p=P)),
                        (w2_sb, d["w2"].rearrange("(ko p) f -> p ko f", p=P))):
                    nc.vector.tensor_copy(out=pt_[0:1, 0, 0:1],
                                          in_=KT[0:1, 0, 0:1])
                    nc.gpsimd.dma_start(pt_, src)
                for m in range(NE):
                    for half in range(2):
                        hs_ = slice(half * TB, (half + 1) * TB)
                        pq = ps_a.tile([P, TB], F32, tag="a", bufs=3)
                        for et in range(NE):
                            nc.tensor.matmul(pq,
                                             wq_sb[:, et, m * P:(m + 1) * P],
                                             xn[:, et, hs_],
                                             start=(et == 0),
                                             stop=(et == NE - 1))
                        nc.scalar.activation(QT[:, m, hs_], pq, AF.Copy)
            # V token-major: one strided copy per 128-token slice
            for ts in range(2 * TB // P):
                pv = ps_a.tile([P, TB], F32, tag="a", bufs=3)
                for et in range(NE):
                    nc.tensor.matmul(pv,
                                     xn[:, et, ts * P:(ts + 1) * P],
                                     wv_sb[:, et, :],
                                     start=(et == 0), stop=(et == NE - 1))
                jt = sb * (2 * TB // P) + ts
                nc.vector.tensor_copy(
                    out=V[:, jt, :, 0:D],
                    in_=pv.rearrange("p (h d) -> p h d", h=H))

    # ------- Phase B: attention (scores^T layout), C interleaved ---------
    with tc.tile_pool(name="pb", bufs=1) as pb, \
            tc.tile_pool(name="rows_b", bufs=1) as rows_b, \
            tc.tile_pool(name="ps_s", bufs=1, space="PSUM") as ps_s, \
            tc.tile_pool(name="ps_o", bufs=1, space="PSUM") as ps_o, \
            tc.tile_pool(name="pc", bufs=1) as pc, \
            tc.tile_pool(name="xc", bufs=2) as xc, \
            tc.tile_pool(name="rows_c", bufs=1) as rows_c, \
            tc.tile_pool(name="bc_c", bufs=1) as bc_c, \
            tc.tile_pool(name="ps_c", bufs=1, space="PSUM") as ps_c:
        X2 = pc.tile([P, NE, LQ], F32)
        X2N = pc.tile([P, NE, LQ], BF16)
        for qb in range(NQB):
            qs = slice(qb * TB, (qb + 1) * TB)
            for pr in range(NPAIR):
                oA = ps_o.tile([P, TB], F32, tag="o", bufs=2)
                oB = ps_o.tile([P, TB], F32, tag="o", bufs=2)
                for jt in range(NJT):
                    js = slice(jt * P, (jt + 1) * P)
                    s0 = ps_s.tile([P, TB], F32, tag="s", bufs=2)
                    s1 = ps_s.tile([P, TB], F32, tag="s", bufs=2)
                    nc.tensor.matmul(s0, KT[0:D, pr, js], QT[0:D, pr, qs],
                                     start=True, stop=True,
                                     tile_position=(0, 0))
                    nc.tensor.matmul(s1, KT[D:P, pr, js], QT[D:P, pr, qs],
                                     start=True, stop=True,
                                     tile_position=(64, 0))
                    pt = pb.tile([P, 2 * TB], BF16, tag="p", bufs=4)
                    # even head: exact exp on ScalarE
                    nc.scalar.activation(pt[:, 0:TB], s0, AF.Exp, scale=SCALE)
                    # odd head: Schraudolph exp in bf16-bit domain on VectorE
                    nc.vector.tensor_scalar(out=pt[:, TB:2 * TB].bitcast(I16),
                                            in0=s1, scalar1=EXP_A,
                                            scalar2=EXP_B,
                                            op0=ALU.mult, op1=ALU.add)
                    nc.tensor.matmul(oA[0:D + 1, :],
                                     V[:, jt, 2 * pr, :],
                                     pt[:, 0:TB],
                                     start=(jt == 0), stop=(jt == NJT - 1))
                    nc.tensor.matmul(oB[0:D + 1, :],
                                     V[:, jt, 2 * pr + 1, :],
                                     pt[:, TB:2 * TB],
                                     start=(jt == 0), stop=(jt == NJT - 1))
                for hi, ops in ((0, oA), (1, oB)):
                    dn = rows_b.tile([1, TB], F32, tag="dn", bufs=4)
                    nc.vector.tensor_copy(out=dn, in_=ops[D:D + 1, :])
                    dr = rows_b.tile([1, TB], F32, tag="dr", bufs=4)
                    nc.vector.reciprocal_approx_fast(dr, dn)
                    db = rows_b.tile([D, TB], F32, tag="db", bufs=4)
                    nc.gpsimd.partition_broadcast(db, dr)
                    nc.vector.tensor_mul(
                        ONORM[hi * D:(hi + 1) * D, pr, qs],
                        ops[0:D, :], db)

            # ---- Phase C for this query half: wo-proj + LN2 + FFN -------
            for m in range(NE):
                pp = ps_c.tile([P, TB], F32, tag="c", bufs=2)
                for et in range(NE):
                    nc.tensor.matmul(pp, wo_sb[:, et, m * P:(m + 1) * P],
                                     ONORM[:, et, qs],
                                     start=(et == 0), stop=(et == NE - 1))
                nc.vector.affine_then_add(X2[:, m, qs], pp,
                                          xt0[:, m, qs], scale=1.0,
                                          bias=vecs[:, VBO + m:VBO + m + 1])
            mu2 = ps_c.tile([1, TB], F32, tag="c", bufs=2)
            ss2 = ps_c.tile([1, TB], F32, tag="c", bufs=2)
            layernorm(rows_c, bc_c, xc,
                      mu2, ss2,
                      [X2[:, et, qs] for et in range(NE)],
                      [X2N[:, et, qs] for et in range(NE)],
                      vecs[:, VG2:VG2 + NE], vecs[:, VB2:VB2 + NE])
            H1 = pc.tile([P, NF1, TB], BF16, tag="h1", bufs=1)
            for m in range(NF1):
                pf = ps_c.tile([P, TB], F32, tag="c", bufs=2)
                for et in range(NE):
                    nc.tensor.matmul(pf, w1_sb[:, et, m * P:(m + 1) * P],
                                     X2N[:, et, qs],
                                     start=(et == 0), stop=(et == NE - 1))
                # silu(u) = u*(1+tanh(u/2))/2, u = pf + bf1
                # (tanh shares the exp activation-table set: no table thrash)
                tg = xc.tile([P, TB], F32, tag="tg", bufs=2)
                nc.scalar.activation(tg, pf, AF.Tanh,
                                     bias=vecs[:, VBF1H + m:VBF1H + m + 1],
                                     scale=0.5)
                hp = xc.tile([P, TB], F32, tag="hp", bufs=2)
                nc.vector.tensor_scalar(out=hp, in0=pf,
                                        scalar1=vecs[:, VBF1 + m:VBF1 + m + 1],
                                        scalar2=0.5,
                                        op0=ALU.add, op1=ALU.mult)
                nc.vector.scalar_tensor_tensor(out=H1[:, m, :], in0=tg,
                                               scalar=1.0, in1=hp,
                                               op0=ALU.add, op1=ALU.mult)
            for m in range(NE):
                po = ps_c.tile([P, TB], F32, tag="c", bufs=2)
                for kt in range(NF1):
                    nc.tensor.matmul(po, w2_sb[:, kt, m * P:(m + 1) * P],
                                     H1[:, kt, :],
                                     start=(kt == 0), stop=(kt == NF1 - 1))
                ot = xc.tile([P, TB], F32, tag="ot", bufs=2)
                nc.vector.affine_then_add(ot, po, X2[:, m, qs],
                                          scale=1.0,
                                          bias=vecs[:, VBF2 + m:VBF2 + m + 1])
                nc.sync.dma_start(outT[m * P:(m + 1) * P, qs], ot)


_CACHE = {}


def _build():
    if "nc" in _CACHE:
        return _CACHE["nc"]
    nc = bacc.Bacc("TRN2", target_bir_lowering=False, debug=False,
                   enable_asserts=False, num_devices=N_CORES)
    d = {
        "vtag": nc.dram_tensor("vtag", [1, KERNEL_VERSION], I32,
                               kind="ExternalInput").ap(),
        "xT": nc.dram_tensor("xT", [E, L], BF16, kind="ExternalInput").ap(),
        "wq": nc.dram_tensor("wq", [E, E], BF16, kind="ExternalInput").ap(),
        "wk": nc.dram_tensor("wk", [E, E], BF16, kind="ExternalInput").ap(),
        "wv": nc.dram_tensor("wv", [E, E], BF16, kind="ExternalInput").ap(),
        "wo": nc.dram_tensor("wo", [E, E], BF16, kind="ExternalInput").ap(),
        "w1": nc.dram_tensor("w1", [E, FF], BF16, kind="ExternalInput").ap(),
        "w2": nc.dram_tensor("w2", [FF, E], BF16, kind="ExternalInput").ap(),
        "vecs": nc.dram_tensor("vecs", [P, VECS_W], F32,
                               kind="ExternalInput").ap(),
        "outT": nc.dram_tensor("outT", [E, LQ], F32,
                               kind="ExternalOutput").ap(),
    }
    with tile.TileContext(nc) as tc:
        _emit(tc, d)
    nc.compile()
    _CACHE["nc"] = nc
    return nc


def _in_maps(inputs):
    import ml_dtypes
    x = np.ascontiguousarray(np.asarray(inputs["x"], dtype=np.float32))
    ws = {}
    for k in ("wq", "wk", "wv", "wo", "w1", "w2"):
        ws[k] = np.ascontiguousarray(
            np.asarray(inputs[k], dtype=np.float32).astype(ml_dtypes.bfloat16))

    def cols(v, n):
        return np.asarray(v, np.float32).reshape(n, P).T

    bf1 = np.asarray(inputs["bf1"], np.float32)
    ws["vecs"] = np.ascontiguousarray(np.concatenate(
        [cols(inputs["g1"], NE), cols(inputs["b1"], NE),
         cols(inputs["g2"], NE), cols(inputs["b2"], NE),
         cols(inputs["bo"], NE), cols(inputs["bf2"], NE),
         cols(bf1, NF1), cols(bf1 * 0.5, NF1)], axis=1))
    maps = []
    for c in range(N_CORES):
        n, hf = c // 2, c % 2
        xp = np.concatenate(
            [x[n, hf * LQ:(hf + 1) * LQ], x[n, (1 - hf) * LQ:(2 - hf) * LQ]],
            axis=0)
        m = dict(ws)
        m["vtag"] = np.zeros((1, KERNEL_VERSION), np.int32)
        m["xT"] = np.ascontiguousarray(xp.T.astype(ml_dtypes.bfloat16))
        maps.append(m)
    return maps


def kernel_with_results(**inputs):
    nc = _build()
    res = run_bass_kernel_spmd(
        nc, _in_maps(inputs), core_ids=list(range(N_CORES)),
        trace=bool(int(os.environ.get("KERNEL_TRACE", "0"))))
    x = np.asarray(inputs["x"])
    out = np.empty((x.shape[0], L, E), dtype=np.float32)
    for c in range(N_CORES):
        n, hf = c // 2, c % 2
        out[n, hf * LQ:(hf + 1) * LQ] = res.results[c]["outT"].T
    return out, res


def kernel(**inputs):
    return kernel_with_results(**inputs)[0]


# revision 10
# speedup vs baseline: 1.1399x; 1.1399x over previous
"""Trainium2 Bass kernel for nn_AttentionBlock (pre-LN transformer block).

Reference math (fp32):
    h  = LN(x, g1, b1); q,k,v = h@wq, h@wk, h@wv  (heads=8, d=64)
    att = softmax(q k^T / sqrt(512)) v;  x2 = att@wo + bo + x
    out = silu(LN(x2,g2,b2)@w1 + bf1)@w2 + bf2 + x2

Sharding over 8 NeuronCores: core c handles batch n=c//2, query-half c%2
(1024 queries). K/V are computed per-core over the full 2048-token sequence
(duplicated within each batch pair; softmax over keys is permutation
invariant, so each core receives the sequence rotated so its own tokens come
first — fixed offsets, single shared NEFF, no collectives).

v11 layout/scheduling notes:
  - feature-major on-chip ([feature partitions, token free-dim]); host passes
    x pre-transposed per core, output transposed back on host.
  - per-feature vectors ride in one packed [128, 56] fp32 "vecs" DMA; the
    residual for x2 reuses the bf16 sb0 input tile (no separate fp32 DMA).
  - phase-C weight prefetches are gated on an early phase-A tile so the Tile
    scheduler cannot hoist them to t=0 and starve the critical-path DMAs.
  - attention/FFN PSUM+SBUF pools are opened BEFORE phase A's transient pools
    so attention for key-tiles 0-7 can overlap phase A's second superblock
    (later-opened pools would inherit release deps on A's addresses).
  - attention runs qb-outer so phase C for query-half 0 overlaps attention of
    half 1; silu uses Tanh (same activation table set as Exp — one table
    load); LN uses the Newton-rsqrt DVE path (no scalar Sqrt table); LN2's
    elementwise work runs on GPSIMD to unload the vector engine.
"""

import os
from contextlib import ExitStack

import numpy as np

import concourse.bass as bass
import concourse.tile as tile
from concourse import bacc, mybir
from concourse._compat import with_exitstack
from concourse.bass_utils import run_bass_kernel_spmd

F32 = mybir.dt.float32
BF16 = mybir.dt.bfloat16
I32 = mybir.dt.int32
I16 = mybir.dt.int16
AF = mybir.ActivationFunctionType
ALU = mybir.AluOpType

P = 128          # partitions
E = 512          # embedding
NE = E // P      # 4 feature tiles
L = 2048         # sequence length
LQ = 1024        # queries per core
TB = 512         # token block
NTB = L // TB    # 4
NQB = LQ // TB   # 2
H = 8            # heads
D = 64           # head dim
NPAIR = H // 2   # 4 head pairs
NJT = L // P     # 16 key tiles
FF = 2048
NF1 = FF // P    # 16
SCALE = float(E) ** -0.5
EPS = 1e-5
MAGIC = 0x5F3759DF
N_CORES = 8
KERNEL_VERSION = 13  # bump on every kernel change: busts the neuron compile cache

# packed vecs column offsets
VG1, VB1, VG2, VB2, VBO, VBF2, VBF1, VBF1H = 0, 4, 8, 12, 16, 20, 24, 40
VECS_W = 56


@with_exitstack
def _emit(ctx: ExitStack, tc: tile.TileContext, d):
    nc = tc.nc
    xT = d["xT"]          # [E, L]
    outT = d["outT"]      # [E, LQ]

    pers = ctx.enter_context(tc.tile_pool(name="pers", bufs=1))

    # constants + packed per-feature vectors
    one_f = pers.tile([P, 1], F32)
    nc.vector.memset(one_f, 1.0)
    inv_e_f = pers.tile([P, 1], F32)
    nc.vector.memset(inv_e_f, 1.0 / E)
    one_col = pers.tile([P, 1], BF16)            # 1/E for mean matmuls
    nc.vector.tensor_copy(out=one_col, in_=inv_e_f)
    magic = pers.tile([1, 1], I32)
    nc.vector.memset(magic, MAGIC)
    vecs = pers.tile([P, VECS_W], F32)
    nc.sync.dma_start(vecs, d["vecs"])

    ONORM = pers.tile([P, NE, LQ], BF16)         # attention out^T, normalized

    def layernorm(rows, bcast, sqpool, mu_ps, ss_ps, x_slices, out_slices,
                  gcol, bcol, eng):
        """x slices: NE aps of [P, TB] (feature-major), fp32 or bf16. Stats via
        ones-matmuls (partition reduction), rstd via DVE Newton rsqrt,
        per-token affine broadcast via GPSIMD. `eng` runs the bulk
        elementwise work (nc.vector in phase A, nc.gpsimd in phase C)."""
        xbs = []
        for et in range(NE):
            if x_slices[et].dtype == BF16:
                xb = x_slices[et]
            else:
                xb = sqpool.tile([P, TB], BF16, tag="xb", bufs=4)
                eng.tensor_copy(out=xb, in_=x_slices[et])
            xbs.append(xb)
            sq = sqpool.tile([P, TB], BF16, tag="sq", bufs=2)
            eng.tensor_tensor(out=sq, in0=xb, in1=xb, op=ALU.mult)
            nc.tensor.matmul(mu_ps, one_col, xb,
                             start=(et == 0), stop=(et == NE - 1))
            nc.tensor.matmul(ss_ps, one_col, sq,
                             start=(et == 0), stop=(et == NE - 1))
        mur = rows.tile([1, TB], F32, tag="mur", bufs=1)
        v = rows.tile([1, TB], F32, tag="var", bufs=1)
        y = rows.tile([1, TB], F32, tag="y", bufs=1)
        t = rows.tile([1, TB], F32, tag="t", bufs=1)
        b = rows.tile([1, TB], F32, tag="b", bufs=1)
        nc.vector.tensor_copy(out=mur, in_=mu_ps)
        nc.vector.tensor_mul(t, mur, mur)
        # v = (ss + EPS) - mu^2
        nc.vector.scalar_tensor_tensor(out=v, in0=ss_ps, scalar=EPS, in1=t,
                                       op0=ALU.add, op1=ALU.subtract)
        # rstd = rsqrt(v): fast-inverse-sqrt seed + Newton steps (DVE only)
        nc.vector.tensor_scalar(out=y.bitcast(I32), in0=v.bitcast(I32),
                                scalar1=1, scalar2=None,
                                op0=ALU.logical_shift_right)
        nc.vector.tensor_tensor(out=y.bitcast(I32),
                                in0=magic.to_broadcast([1, TB]),
                                in1=y.bitcast(I32), op=ALU.subtract)
        for _ in range(2):
            nc.vector.tensor_mul(t, y, y)
            nc.vector.tensor_mul(t, t, v)
            nc.vector.tensor_scalar(out=t, in0=t, scalar1=-0.5,
                                    scalar2=1.5,
                                    op0=ALU.mult, op1=ALU.add)
            nc.vector.tensor_mul(y, y, t)
        nc.vector.tensor_mul(b, mur, y)
        yb = rows.tile([1, TB], BF16, tag="yb", bufs=1)
        bb = rows.tile([1, TB], BF16, tag="bb16", bufs=1)
        nc.vector.tensor_copy(out=yb, in_=y)
        nc.vector.tensor_copy(out=bb, in_=b)
        a_b = bcast.tile([P, TB], BF16, tag="ab", bufs=2)
        b_b = bcast.tile([P, TB], BF16, tag="bb", bufs=2)
        nc.gpsimd.partition_broadcast(a_b, yb)
        nc.gpsimd.partition_broadcast(b_b, bb)
        for et in range(NE):
            o = out_slices[et]          # bf16; all-bf16 ops for 2x DVE
            sc = sqpool.tile([P, TB], BF16, tag="lnsc", bufs=2)
            eng.tensor_tensor(out=sc, in0=xbs[et], in1=a_b, op=ALU.mult)
            eng.tensor_tensor(out=sc, in0=sc, in1=b_b, op=ALU.subtract)
            eng.tensor_scalar(out=o, in0=sc,
                              scalar1=gcol[:, et:et + 1],
                              scalar2=bcol[:, et:et + 1],
                              op0=ALU.mult, op1=ALU.add)

    # Phase-C operands, DMA'd mid-phase-A (gated below so the scheduler
    # cannot start them at t=0 ahead of the critical-path DMAs)
    pre = ctx.enter_context(tc.tile_pool(name="prefetch", bufs=1))
    wo_sb = pre.tile([P, NE, E], BF16)
    w1_sb = pre.tile([P, NE, FF], BF16)
    w2_sb = pre.tile([P, NF1, E], BF16)

    # persistent attention tensors + pools that must pre-date phase A's
    # transient pools (so attention can overlap phase A without inheriting
    # release deps on reused addresses)
    pab = ctx.enter_context(tc.tile_pool(name="qkv_pers", bufs=1))
    KT = pab.tile([P, NE, L], BF16)          # K^T  (feature-major)
    QT = pab.tile([P, NE, LQ], BF16)         # Q^T
    V = pab.tile([P, NJT, H, D + 1], BF16)   # V token-major + ones column
    xt0 = pab.tile([P, NE, 2 * TB], BF16)    # sb0 tokens: LN1 input + residual
    XR = pab.tile([P, NE, LQ], F32)          # fp32 view of xt0 for residuals
    nc.vector.tensor_copy(out=V[:, :, :, D:D + 1],
                          in_=one_f.unsqueeze(1).unsqueeze(1)
                          .to_broadcast([P, NJT, H, 1]))
    pb = ctx.enter_context(tc.tile_pool(name="pb", bufs=1))
    rows_b = ctx.enter_context(tc.tile_pool(name="rows_b", bufs=1))
    ps_s = ctx.enter_context(tc.tile_pool(name="ps_s", bufs=1, space="PSUM"))
    ps_o = ctx.enter_context(tc.tile_pool(name="ps_o", bufs=1, space="PSUM"))

    # ---------------- Phase A: LN1 + Q/K/V projections -------------------
    with tc.tile_pool(name="wa", bufs=1) as wa, \
            tc.tile_pool(name="xa", bufs=2) as xa, \
            tc.tile_pool(name="rows_a", bufs=1) as rows_a, \
            tc.tile_pool(name="bc_a", bufs=1) as bc_a, \
            tc.tile_pool(name="ps_a", bufs=1, space="PSUM") as ps_a:
        wq_sb = wa.tile([P, NE, E], BF16, tag="wq")
        wk_sb = wa.tile([P, NE, E], BF16, tag="wk")
        wv_sb = wa.tile([P, NE, E], BF16, tag="wv")
        for t, name in ((wk_sb, "wk"), (wq_sb, "wq"), (wv_sb, "wv")):
            nc.sync.dma_start(
                t, d[name].rearrange("(ko p) f -> p ko f", p=P))

        for sb in range(NTB // 2):      # super-blocks of 1024 tokens
            xn = xa.tile([P, NE, 2 * TB], BF16, tag="xn", bufs=2)
            xt = xt0 if sb == 0 else xa.tile([P, NE, 2 * TB], BF16,
                                             tag="xa", bufs=1)
            nc.sync.dma_start(
                xt, xT[:, sb * 2 * TB:(sb + 1) * 2 * TB].rearrange(
                    "(et p) t -> p et t", p=P))
            for half in range(2):
                hs_ = slice(half * TB, (half + 1) * TB)
                mu_ps = ps_a.tile([1, TB], F32, tag="a", bufs=2)
                ss_ps = ps_a.tile([1, TB], F32, tag="a", bufs=2)
                layernorm(rows_a, bc_a, xa, mu_ps, ss_ps,
                          [xt[:, et, hs_] for et in range(NE)],
                          [xn[:, et, hs_] for et in range(NE)],
                          vecs[:, VG1:VG1 + NE], vecs[:, VB1:VB1 + NE],
                          nc.vector)
            if sb == 0:
                nc.vector.tensor_copy(out=XR, in_=xt0)
            # K^T (and Q^T for this core's token super-block)
            for m in range(NE):
                for half in range(2):
                    hs_ = slice(half * TB, (half + 1) * TB)
                    pk = ps_a.tile([P, TB], F32, tag="a", bufs=2)
                    for et in range(NE):
                        nc.tensor.matmul(pk, wk_sb[:, et, m * P:(m + 1) * P],
                                         xn[:, et, hs_],
                                         start=(et == 0), stop=(et == NE - 1))
                    ks_ = slice(sb * 2 * TB + half * TB,
                                sb * 2 * TB + (half + 1) * TB)
                    nc.scalar.activation(KT[:, m, ks_], pk, AF.Copy)
            if sb == 0:
                # gated phase-C prefetches: the tiny dependency write keeps
                # the 4.5MB of DMA off the t=0 critical path
                for pt_, src in (
                        (wo_sb, d["wo"].rearrange("(ko p) f -> p ko f", p=P)),
                        (w1_sb, d["w1"].rearrange("(ko p) f -> p ko f", p=P)),
                        (w2_sb, d["w2"].rearrange("(ko p) f -> p ko f", p=P))):
                    nc.vector.tensor_copy(out=pt_[0:1, 0, 0:1],
                                          in_=KT[0:1, 0, 0:1])
                    nc.gpsimd.dma_start(pt_, src)
                for m in range(NE):
                    for half in range(2):
                        hs_ = slice(half * TB, (half + 1) * TB)
                        pq = ps_a.tile([P, TB], F32, tag="a", bufs=2)
                        for et in range(NE):
                            nc.tensor.matmul(pq,
                                             wq_sb[:, et, m * P:(m + 1) * P],
                                             xn[:, et, hs_],
                                             start=(et == 0),
                                             stop=(et == NE - 1))
                        nc.scalar.activation(QT[:, m, hs_], pq, AF.Copy)
            # V token-major: one strided copy per 128-token slice
            for ts in range(2 * TB // P):
                pv = ps_a.tile([P, TB], F32, tag="a", bufs=2)
                for et in range(NE):
                    nc.tensor.matmul(pv,
                                     xn[:, et, ts * P:(ts + 1) * P],
                                     wv_sb[:, et, :],
                                     start=(et == 0), stop=(et == NE - 1))
                jt = sb * (2 * TB // P) + ts
                nc.scalar.activation(
                    V[:, jt, :, 0:D],
                    pv.rearrange("p (h d) -> p h d", h=H), AF.Copy)

    # ------- Phase B: attention (scores^T layout), C interleaved ---------
    with tc.tile_pool(name="pc", bufs=1) as pc, \
            tc.tile_pool(name="xc", bufs=2) as xc, \
            tc.tile_pool(name="rows_c", bufs=1) as rows_c, \
            tc.tile_pool(name="bc_c", bufs=1) as bc_c, \
            tc.tile_pool(name="ps_c", bufs=1, space="PSUM") as ps_c:
        X2 = pc.tile([P, NE, LQ], F32)
        X2N = pc.tile([P, NE, LQ], BF16)
        for qb in range(NQB):
            qs = slice(qb * TB, (qb + 1) * TB)
            for pr in range(NPAIR):
                oA = ps_o.tile([P, TB], F32, tag="o", bufs=2)
                oB = ps_o.tile([P, TB], F32, tag="o", bufs=2)
                for jt in range(NJT):
                    js = slice(jt * P, (jt + 1) * P)
                    s = ps_s.tile([P, 2 * TB], F32, tag="s", bufs=2)
                    nc.tensor.matmul(s[:, 0:TB], KT[0:D, pr, js],
                                     QT[0:D, pr, qs],
                                     start=True, stop=True,
                                     tile_position=(0, 0))
                    nc.tensor.matmul(s[:, TB:2 * TB], KT[D:P, pr, js],
                                     QT[D:P, pr, qs],
                                     start=True, stop=True,
                                     tile_position=(64, 0))
                    pt = pb.tile([P, 2 * TB], BF16, tag="p", bufs=4)
                    nc.scalar.activation(pt, s, AF.Exp, scale=SCALE)
                    nc.tensor.matmul(oA[0:D + 1, :],
                                     V[:, jt, 2 * pr, :],
                                     pt[:, 0:TB],
                                     start=(jt == 0), stop=(jt == NJT - 1))
                    nc.tensor.matmul(oB[0:D + 1, :],
                                     V[:, jt, 2 * pr + 1, :],
                                     pt[:, TB:2 * TB],
                                     start=(jt == 0), stop=(jt == NJT - 1))
                for hi, ops in ((0, oA), (1, oB)):
                    dn = rows_b.tile([1, TB], F32, tag="dn", bufs=2)
                    nc.vector.tensor_copy(out=dn, in_=ops[D:D + 1, :])
                    dr = rows_b.tile([1, TB], F32, tag="dr", bufs=2)
                    nc.vector.reciprocal_approx_fast(dr, dn)
                    db = rows_b.tile([D, TB], F32, tag="db", bufs=2)
                    nc.gpsimd.partition_broadcast(db, dr)
                    nc.vector.tensor_mul(
                        ONORM[hi * D:(hi + 1) * D, pr, qs],
                        ops[0:D, :], db)

            # ---- Phase C for this query half: wo-proj + LN2 + FFN -------
            for m in range(NE):
                pp = ps_c.tile([P, TB], F32, tag="c", bufs=2)
                for et in range(NE):
                    nc.tensor.matmul(pp, wo_sb[:, et, m * P:(m + 1) * P],
                                     ONORM[:, et, qs],
                                     start=(et == 0), stop=(et == NE - 1))
                nc.vector.affine_then_add(X2[:, m, qs], pp,
                                          XR[:, m, qs], scale=1.0,
                                          bias=vecs[:, VBO + m:VBO + m + 1])
            mu2 = ps_c.tile([1, TB], F32, tag="c", bufs=2)
            ss2 = ps_c.tile([1, TB], F32, tag="c", bufs=2)
            layernorm(rows_c, bc_c, xc, mu2, ss2,
                      [X2[:, et, qs] for et in range(NE)],
                      [X2N[:, et, qs] for et in range(NE)],
                      vecs[:, VG2:VG2 + NE], vecs[:, VB2:VB2 + NE],
                      nc.gpsimd)
            H1 = pc.tile([P, NF1, TB], BF16, tag="h1", bufs=1)
            for m in range(NF1):
                pf = ps_c.tile([P, TB], F32, tag="c", bufs=2)
                for et in range(NE):
                    nc.tensor.matmul(pf, w1_sb[:, et, m * P:(m + 1) * P],
                                     X2N[:, et, qs],
                                     start=(et == 0), stop=(et == NE - 1))
                # silu(u) = u*(1+tanh(u/2))/2, u = pf + bf1
                # (tanh shares the exp activation-table set: no table thrash)
                tg = xc.tile([P, TB], F32, tag="tg", bufs=2)
                nc.scalar.activation(tg, pf, AF.Tanh,
                                     bias=vecs[:, VBF1H + m:VBF1H + m + 1],
                                     scale=0.5)
                hp = xc.tile([P, TB], F32, tag="hp", bufs=2)
                nc.vector.tensor_scalar(out=hp, in0=pf,
                                        scalar1=vecs[:, VBF1 + m:VBF1 + m + 1],
                                        scalar2=0.5,
                                        op0=ALU.add, op1=ALU.mult)
                nc.vector.scalar_tensor_tensor(out=H1[:, m, :], in0=tg,
                                               scalar=1.0, in1=hp,
                                               op0=ALU.add, op1=ALU.mult)
            for m in range(NE):
                po = ps_c.tile([P, TB], F32, tag="c", bufs=2)
                for kt in range(NF1):
                    nc.tensor.matmul(po, w2_sb[:, kt, m * P:(m + 1) * P],
                                     H1[:, kt, :],
                                     start=(kt == 0), stop=(kt == NF1 - 1))
                ot = xc.tile([P, TB], F32, tag="ot", bufs=2)
                nc.vector.affine_then_add(ot, po, X2[:, m, qs],
                                          scale=1.0,
                                          bias=vecs[:, VBF2 + m:VBF2 + m + 1])
                nc.sync.dma_start(outT[m * P:(m + 1) * P, qs], ot)


_CACHE = {}


def _build():
    if "nc" in _CACHE:
        return _CACHE["nc"]
    nc = bacc.Bacc("TRN2", target_bir_lowering=False, debug=False,
                   enable_asserts=False, num_devices=N_CORES)
    d = {
        "vtag": nc.dram_tensor("vtag", [1, KERNEL_VERSION], I32,
                               kind="ExternalInput").ap(),
        "xT": nc.dram_tensor("xT", [E, L], BF16, kind="ExternalInput").ap(),
        "wq": nc.dram_tensor("wq", [E, E], BF16, kind="ExternalInput").ap(),
        "wk": nc.dram_tensor("wk", [E, E], BF16, kind="ExternalInput").ap(),
        "wv": nc.dram_tensor("wv", [E, E], BF16, kind="ExternalInput").ap(),
        "wo": nc.dram_tensor("wo", [E, E], BF16, kind="ExternalInput").ap(),
        "w1": nc.dram_tensor("w1", [E, FF], BF16, kind="ExternalInput").ap(),
        "w2": nc.dram_tensor("w2", [FF, E], BF16, kind="ExternalInput").ap(),
        "vecs": nc.dram_tensor("vecs", [P, VECS_W], F32,
                               kind="ExternalInput").ap(),
        "outT": nc.dram_tensor("outT", [E, LQ], F32,
                               kind="ExternalOutput").ap(),
    }
    with tile.TileContext(nc) as tc:
        _emit(tc, d)
    nc.compile()
    _CACHE["nc"] = nc
    return nc


def _in_maps(inputs):
    import ml_dtypes
    x = np.ascontiguousarray(np.asarray(inputs["x"], dtype=np.float32))
    ws = {}
    for k in ("wq", "wk", "wv", "wo", "w1", "w2"):
        ws[k] = np.ascontiguousarray(
            np.asarray(inputs[k], dtype=np.float32).astype(ml_dtypes.bfloat16))

    def cols(v, n):
        return np.asarray(v, np.float32).reshape(n, P).T

    bf1 = np.asarray(inputs["bf1"], np.float32)
    ws["vecs"] = np.ascontiguousarray(np.concatenate(
        [cols(inputs["g1"], NE), cols(inputs["b1"], NE),
         cols(inputs["g2"], NE), cols(inputs["b2"], NE),
         cols(inputs["bo"], NE), cols(inputs["bf2"], NE),
         cols(bf1, NF1), cols(bf1 * 0.5, NF1)], axis=1))
    maps = []
    for c in range(N_CORES):
        n, hf = c // 2, c % 2
        xp = np.concatenate(
            [x[n, hf * LQ:(hf + 1) * LQ], x[n, (1 - hf) * LQ:(2 - hf) * LQ]],
            axis=0)
        m = dict(ws)
        m["vtag"] = np.zeros((1, KERNEL_VERSION), np.int32)
        m["xT"] = np.ascontiguousarray(xp.T.astype(ml_dtypes.bfloat16))
        maps.append(m)
    return maps


def kernel_with_results(**inputs):
    nc = _build()
    res = run_bass_kernel_spmd(
        nc, _in_maps(inputs), core_ids=list(range(N_CORES)),
        trace=bool(int(os.environ.get("KERNEL_TRACE", "0"))))
    x = np.asarray(inputs["x"])
    out = np.empty((x.shape[0], L, E), dtype=np.float32)
    for c in range(N_CORES):
        n, hf = c // 2, c % 2
        out[n, hf * LQ:(hf + 1) * LQ] = res.results[c]["outT"].T
    return out, res


def kernel(**inputs):
    return kernel_with_results(**inputs)[0]
